# revision 2
# baseline (speedup 1.0000x reference)
"""Trainium2 Bass kernel for a ByteNet-style Markov LM over sliding windows.

x (8, 2048) int tokens -> emb windows (B*W, 512, 9) -> 2 ByteNet layers
(inorm+gelu, 1x1 512->256, inorm+gelu, k=5 conv 256->256, inorm+gelu,
1x1 256->512, residual) -> flatten -> (4608 -> 7) projection -> (8, 2040, 7).

Sharding: pure data parallel, one batch row per NeuronCore (8 cores).

v3 design:
  - Software-pipelined wavefront emission (13 stages, ~13 tiles in flight)
    so every engine's FIFO interleaves independent tiles.
  - gamma=1/beta=0/out_b=0 asserted on host and dropped from the kernel.
  - Layer-0 norm1 stats via global sliding 9-sums S (and S2 of squares)
    precomputed once over the position axis (windows overlap; per-tile cost
    is a handful of small ops instead of big reduces).
  - fp16 packed-SBUF elementwise (DVE 2x modes); per-op-site engine knobs
    (DVE / Pool / ACT-Identity) for load balancing, tuned via the cost sim.
  - rsqrt via fp32 magic seed + 1 Newton step (max 0.18% rel err).
  - sq scratch written into the norm's output tile (dead until affine).
  - Output projection transposed: ow stationary (7 cols), h moving;
    (7, 51) PSUM tiles -> (7, 2040) fp16 SBUF strip -> one DMA -> host
    transpose.
"""

import os
from contextlib import ExitStack

import numpy as np

os.environ.setdefault("MYCRO_LOCAL_CACHE", "1")

import concourse.bass as bass
import concourse.bacc as bacc
import concourse.mybir as mybir
from concourse import tile
from concourse.bass_utils import run_bass_kernel_spmd

FP = mybir.dt.float32
U32 = mybir.dt.uint32
RT = mybir.dt.float16
NPRT = np.float16
AF = mybir.ActivationFunctionType
ALU = mybir.AluOpType
AX = mybir.AxisListType

K = 9
VOCAB = 7
DIM = 512
LOW = 256
LSEQ = 2048
B = 8
W = LSEQ - K + 1  # 2040
NW = 51           # windows per tile
NT = W // NW      # 40 tiles
F = K * NW        # 459 free elements per (cb, tile)
NL = 2
EPS = 1e-5
MAGIC = float(0x5f3759df)

NCB_HI = DIM // 128   # 4
NCB_LO = LOW // 128   # 2

# Engine assignment knobs (tuned against the cost-model sim).
CFG = {
    "sq_hi": "dve",
    "sq_lo": "dve",
    "ssq_hi": "dtree",
    "ssq_lo": "dtree",
    "ns_hi": "dtree",
    "aff_hi": "dve",
    "aff_lo": "dve",
    "z_hi": "dve",
    "z_lo": "dve",
    "qv": "pool",
    "wv": "act",
    "y0f": "pool",
    "ysq": "act",
    "yc": "act",     # fp->int convert: act|dve|pool
    "n23": "id9",    # norm2/3 mean handling: id9|raw
    "ns_lo": "ptree",  # raw-form window sums: dred|dtree|ptree
    "tree1": "dve",  # tree stage-1 engine: dve|pool
    "u": "dve",      # newton u = ysq*qv: dve|pool
    "s16": "dve",    # s16 = y*wv: dve|pool
    "evac_h": "act",  # mm3 PSUM evac: act|dve
    "evac_z": "act",  # norm2/3 z evac: act|dve
    "pair": "off",   # cross-layer chain pairing
}


def v3(base, off, dims):
    """View of a 2D (P, F) AP with explicit free dims [[step, count], ...]."""
    return bass.AP(base.tensor, base.offset + off, [list(base.ap[0])] + [list(d) for d in dims])


def build(n_tiles=NT, cfg=None):
    cfg = dict(CFG if cfg is None else cfg)
    nc = bacc.Bacc("TRN2", target_bir_lowering=False, debug=False)

    # ---- DRAM I/O ----
    oneh_d = nc.dram_tensor("oneh", [VOCAB, LSEQ], RT, kind="ExternalInput")
    emb_d = nc.dram_tensor("embw", [VOCAB, DIM], RT, kind="ExternalInput")
    w1_d = nc.dram_tensor("w1", [NL, NCB_HI, 128, LOW], RT, kind="ExternalInput")
    w2_d = nc.dram_tensor("w2", [NL, 5, NCB_LO, 128, LOW], RT, kind="ExternalInput")
    w3_d = nc.dram_tensor("w3", [NL, NCB_LO, 128, DIM], RT, kind="ExternalInput")
    ow_d = nc.dram_tensor("ow", [NCB_HI, K, 128, VOCAB], RT, kind="ExternalInput")
    id1_d = nc.dram_tensor("id1", [128, 128], RT, kind="ExternalInput")
    id9_d = nc.dram_tensor("id9", [128, 128], RT, kind="ExternalInput")
    out_d = nc.dram_tensor("out", [VOCAB, W], RT, kind="ExternalOutput")

    with tile.TileContext(nc) as tc, ExitStack() as ctx:
        const = ctx.enter_context(tc.tile_pool(name="const", bufs=1))
        work = ctx.enter_context(tc.tile_pool(name="work", bufs=int(os.environ.get("WB", "3"))))
        stat = ctx.enter_context(tc.tile_pool(name="stat", bufs=int(os.environ.get("SB", "4"))))
        rings = cfg.get("rings", (2, 2, 2, 2))
        psm1 = ctx.enter_context(tc.tile_pool(name="psm1", bufs=rings[0], space="PSUM"))
        pscv = ctx.enter_context(tc.tile_pool(name="pscv", bufs=rings[1], space="PSUM"))
        psm3 = ctx.enter_context(tc.tile_pool(name="psm3", bufs=rings[2], space="PSUM"))
        pso = ctx.enter_context(tc.tile_pool(name="pso", bufs=rings[3], space="PSUM"))

        def TT(e):
            return nc.vector if e == "dve" else nc.gpsimd

        # ---- load constants ----
        onehsb = const.tile([VOCAB, LSEQ], RT)
        nc.sync.dma_start(onehsb[:, :], oneh_d[:, :])
        embsb = const.tile([VOCAB, DIM], RT)
        nc.sync.dma_start(embsb[:, :], emb_d[:, :])

        w1sb, w2sb, w3sb = [], [], []
        for i in range(NL):
            t1 = const.tile([128, NCB_HI * LOW], RT, name=f"w1sb{i}")
            for kb in range(NCB_HI):
                nc.sync.dma_start(t1[:, kb * LOW:(kb + 1) * LOW], w1_d[i, kb])
            w1sb.append(t1)
            t2 = const.tile([128, 5 * NCB_LO * LOW], RT, name=f"w2sb{i}")
            for d in range(5):
                for kb in range(NCB_LO):
                    j = d * NCB_LO + kb
                    nc.sync.dma_start(t2[:, j * LOW:(j + 1) * LOW], w2_d[i, d, kb])
            w2sb.append(t2)
            t3 = const.tile([128, NCB_LO * DIM], RT, name=f"w3sb{i}")
            for kb in range(NCB_LO):
                nc.sync.dma_start(t3[:, kb * DIM:(kb + 1) * DIM], w3_d[i, kb])
            w3sb.append(t3)

        owsb = const.tile([128, NCB_HI * K * VOCAB], RT)
        for cb in range(NCB_HI):
            for t in range(K):
                j = cb * K + t
                nc.sync.dma_start(owsb[:, j * VOCAB:(j + 1) * VOCAB], ow_d[cb, t])

        id1sb = const.tile([128, 128], RT)
        nc.sync.dma_start(id1sb[:, :], id1_d[:, :])
        id9sb = const.tile([128, 128], RT)
        nc.sync.dma_start(id9sb[:, :], id9_d[:, :])

        # output strip (7 partitions x W) in fp16
        osb = const.tile([VOCAB, W], RT)

        epsb = const.tile([128, 1], FP)
        nc.gpsimd.memset(epsb[:, :], float(EPS))
        c15b = const.tile([128, 1], FP)
        nc.gpsimd.memset(c15b[:, :], 1.5)

        # ---- embedding: eTall (128, 4*LSEQ); block cb at cols [cb*LSEQ, ...) ----
        eTall = const.tile([128, NCB_HI * LSEQ], RT)
        for cb in range(NCB_HI):
            for ch in range(LSEQ // 512):
                pe_ps = psm1.tile([128, 512], FP, tag="pm1", name="pe_ps")
                nc.tensor.matmul(
                    pe_ps[:, :],
                    embsb[:, cb * 128:(cb + 1) * 128],
                    onehsb[:, ch * 512:(ch + 1) * 512],
                    start=True, stop=True,
                )
                nc.scalar.copy(eTall[:, cb * LSEQ + ch * 512: cb * LSEQ + ch * 512 + 512],
                               pe_ps[:, :])

        # ---- global sliding 9-sums over positions: S (sums), S2 (sums of sq) ----
        # S[cb, p] = sum_{t<9} eT[cb, p+t], valid p in [0, W).
        Ssb = const.tile([128, NCB_HI * LSEQ], RT)
        S2sb = const.tile([128, NCB_HI * LSEQ], RT)
        with nc.allow_low_precision("fp16 sliding sums drive mean/var"):
            for cb in range(NCB_HI):
                o = cb * LSEQ
                e2c = work.tile([128, LSEQ], RT, tag="pre_e", name="e2c", bufs=1)
                nc.vector.tensor_mul(e2c[:, :], eTall[:, o:o + LSEQ], eTall[:, o:o + LSEQ])
                for (src, srco, dst) in ((eTall, o, Ssb), (e2c, 0, S2sb)):
                    ac = work.tile([128, LSEQ], RT, tag="pre_a", name="ac", bufs=1)
                    n3 = LSEQ - 6
                    nc.vector.tensor_tensor(
                        ac[:, :n3], v3(src[:, :], srco, [[1, n3]]),
                        v3(src[:, :], srco + 3, [[1, n3]]), op=ALU.add)
                    nc.gpsimd.tensor_tensor(
                        ac[:, :n3], ac[:, :n3],
                        v3(src[:, :], srco + 6, [[1, n3]]), op=ALU.add)
                    nc.vector.tensor_tensor(
                        dst[:, o:o + W], ac[:, :W], ac[:, 1:W + 1], op=ALU.add)
                    nc.gpsimd.tensor_tensor(
                        dst[:, o:o + W], dst[:, o:o + W], ac[:, 2:W + 2], op=ALU.add)

        # ---- helpers ----
        def tree9(e, src_base, src_off, ncb, out_ap, tag):
            """out (128, ncb*NW) fp16 = per-window sums over t of a contiguous
            (128, ncb*F) fp16 region via packed tensor-adds (engine e)."""
            a = work.tile([128, ncb * 3 * NW], RT, tag=f"tr{tag}", name=f"tr{tag}", bufs=3)
            TN = 3 * NW
            e1 = cfg["tree1"]
            av = v3(a[:, :], 0, [[TN, ncb], [1, TN]])
            with nc.allow_low_precision("fp16 window sums"):
                TT(e1).tensor_tensor(
                    av, v3(src_base, src_off, [[F, ncb], [1, TN]]),
                    v3(src_base, src_off + TN, [[F, ncb], [1, TN]]), op=ALU.add)
                TT(e1).tensor_tensor(
                    av, av, v3(src_base, src_off + 2 * TN, [[F, ncb], [1, TN]]),
                    op=ALU.add)
                TT(e).tensor_tensor(
                    out_ap, v3(a[:, :], 0, [[TN, ncb], [1, NW]]),
                    v3(a[:, :], NW, [[TN, ncb], [1, NW]]), op=ALU.add)
                TT(e).tensor_tensor(
                    out_ap, out_ap, v3(a[:, :], 2 * NW, [[TN, ncb], [1, NW]]),
                    op=ALU.add)

        def chain_rsqrt(ssq_ap, n, tagp):
            """s16 (128, n) fp16 = rsqrt(ssq/9 + eps), magic + 1 Newton."""
            qvt = stat.tile([128, n], FP, tag="qv", name=f"qv{tagp}")
            if cfg["qv"] == "act":
                nc.scalar.activation(qvt[:, :], ssq_ap, AF.Identity,
                                     bias=epsb[:, :], scale=1.0 / 9.0)
            elif cfg["qv"] == "dve":
                nc.vector.tensor_scalar(qvt[:, :], ssq_ap, 1.0 / 9.0, float(EPS),
                                        op0=ALU.mult, op1=ALU.add)
            else:
                nc.gpsimd.tensor_scalar(qvt[:, :], ssq_ap, 1.0 / 9.0, float(EPS),
                                        op0=ALU.mult, op1=ALU.add)
            qv = qvt[:, :]
            y0f = stat.tile([128, n], FP, tag="y0f", name=f"y0f{tagp}")
            if cfg["y0f"] == "dve":
                nc.vector.tensor_scalar(y0f[:, :], qv.bitcast(U32), -0.5,
                                        MAGIC, op0=ALU.mult, op1=ALU.add)
            else:
                nc.gpsimd.tensor_scalar(y0f[:, :], qv.bitcast(U32), -0.5,
                                        MAGIC, op0=ALU.mult, op1=ALU.add)
            yc = stat.tile([128, n], FP, tag="yc", name=f"yc{tagp}")
            if cfg["yc"] == "act":
                nc.scalar.copy(yc[:, :].bitcast(U32), y0f[:, :])  # fp value -> bits
            elif cfg["yc"] == "dve":
                nc.vector.tensor_copy(yc[:, :].bitcast(U32), y0f[:, :])
            else:
                nc.gpsimd.tensor_copy(yc[:, :].bitcast(U32), y0f[:, :])
            y = yc[:, :].bitcast(FP)
            ysq = stat.tile([128, n], FP, tag="ysq", name=f"ysq{tagp}")
            if cfg["ysq"] == "act":
                nc.scalar.activation(ysq[:, :], y, AF.Square)
            else:
                nc.vector.tensor_mul(ysq[:, :], y, y)
            # u = ysq*qv; wv = 1.5 - 0.5*u  (in-place on ysq)
            TT(cfg["u"]).tensor_mul(ysq[:, :], ysq[:, :], qv)
            if cfg["wv"] == "act":
                nc.scalar.activation(ysq[:, :], ysq[:, :], AF.Identity,
                                     bias=c15b[:, :], scale=-0.5)
            elif cfg["wv"] == "dve":
                nc.vector.tensor_scalar(ysq[:, :], ysq[:, :], -0.5, 1.5,
                                        op0=ALU.mult, op1=ALU.add)
            else:
                nc.gpsimd.tensor_scalar(ysq[:, :], ysq[:, :], -0.5, 1.5,
                                        op0=ALU.mult, op1=ALU.add)
            s16 = stat.tile([128, n], RT, tag="s16", name=f"s16{tagp}")
            with nc.allow_low_precision("norm scale in fp16"):
                TT(cfg["s16"]).tensor_mul(s16[:, :], y, ysq[:, :])
            return s16

        def sq_ssq(z, ncb, scratch, kind, tagp, ssq, half):
            """Write per-window sums of z^2 into half of the paired ssq tile."""
            e_sq = cfg["sq_hi" if ncb == NCB_HI else "sq_lo"]
            sqv = scratch[:, :ncb * F]
            zf = z[:, :ncb * F]
            with nc.allow_low_precision("z^2 in fp16"):
                TT(e_sq).tensor_mul(sqv, zf, zf)
            off = half * ncb * NW
            how = cfg[kind]
            if how == "dred":
                with nc.allow_low_precision("fp16 ssq"):
                    nc.vector.tensor_reduce(
                        v3(ssq[:, :], off, [[NW, ncb], [1, NW]]),
                        v3(scratch[:, :], 0, [[F, ncb], [1, NW], [NW, K]]),
                        axis=AX.X, op=ALU.add,
                    )
            else:
                tree9("dve" if how == "dtree" else "pool", scratch[:, :], 0, ncb,
                      v3(ssq[:, :], off, [[NW, ncb], [1, NW]]), "s")

        def finish_norm(z, s16, ncb, out_tile, e_aff):
            """out = gelu(z * bcast(s16)); z fp16 (128, ncb*F) SBUF."""
            zv = v3(z[:, :], 0, [[F, ncb], [NW, K], [1, NW]])
            ov = v3(out_tile[:, :], 0, [[F, ncb], [NW, K], [1, NW]])
            sb = v3(s16[:, :], 0, [[NW, ncb], [0, K], [1, NW]])
            with nc.allow_low_precision("normalized activations in fp16"):
                TT(e_aff).tensor_mul(ov, zv, sb)
            nc.scalar.activation(out_tile[:, :ncb * F], out_tile[:, :ncb * F],
                                 AF.Gelu)

        def norm_l0(ti, out_tile):
            """Layer-0 norm1: stats from the global sliding sums S/S2."""
            w0 = ti * NW
            ncb = NCB_HI
            n = ncb * NW
            tagp = "n1l0"
            mh = stat.tile([128, n], RT, tag="mh", name="mh0", bufs=6)
            with nc.allow_low_precision("mean in fp16"):
                nc.vector.tensor_scalar(
                    mh[:, :], v3(Ssb[:, :], w0, [[LSEQ, ncb], [1, NW]]),
                    1.0 / 9.0, None, op0=ALU.mult)
            # z = x - bcast(mh); (cb, t, w) layout matches the F flat order
            z = work.tile([128, ncb * F], RT, tag="z1", name="z1")
            with nc.allow_low_precision("centered activations fp16"):
                TT(cfg["z_hi"]).tensor_tensor(
                    v3(z[:, :], 0, [[F, ncb], [NW, K], [1, NW]]),
                    v3(eTall[:, :], w0, [[LSEQ, ncb], [1, K], [1, NW]]),
                    v3(mh[:, :], 0, [[NW, ncb], [0, K], [1, NW]]),
                    op=ALU.subtract)

            def write_half(ssq, half):
                # ssq half = Sigma z^2 = S2 - 9*m^2 (fp16 smalls)
                off = half * n
                m2 = stat.tile([128, n], RT, tag="mh", name="m2l0", bufs=6)
                with nc.allow_low_precision("m^2 in fp16"):
                    nc.vector.tensor_mul(m2[:, :], mh[:, :], mh[:, :])
                    nc.vector.tensor_scalar(m2[:, :], m2[:, :], -9.0, None,
                                            op0=ALU.mult)
                    nc.vector.tensor_tensor(
                        v3(ssq[:, :], off, [[1, n]]),
                        v3(S2sb[:, :], w0, [[LSEQ, ncb], [1, NW]]),
                        m2[:, :], op=ALU.add)
            return z, write_half

        def norm_sbuf_l1(h_in, out_tile):
            """Layer-1 norm1 on SBUF fp16 h (128, ncb_hi*F)."""
            ncb = NCB_HI
            tagp = "n1l1"
            ns = stat.tile([128, ncb * NW], RT, tag="ns", name="ns1", bufs=6)
            how = cfg["ns_hi"]
            if how == "dred":
                with nc.allow_low_precision("fp16 window sums"):
                    nc.vector.tensor_reduce(
                        v3(ns[:, :], 0, [[NW, ncb], [1, NW]]),
                        v3(h_in[:, :], 0, [[F, ncb], [1, NW], [NW, K]]),
                        axis=AX.X, op=ALU.add,
                    )
            else:
                tree9("dve" if how == "dtree" else "pool", h_in[:, :], 0, ncb,
                      v3(ns[:, :], 0, [[NW, ncb], [1, NW]]), "n")
            mh = stat.tile([128, ncb * NW], RT, tag="mh", name="mh1", bufs=6)
            with nc.allow_low_precision("mean in fp16"):
                nc.vector.tensor_scalar(mh[:, :], ns[:, :], 1.0 / 9.0, None,
                                        op0=ALU.mult)
            z = work.tile([128, ncb * F], RT, tag="z1", name="z1b")
            with nc.allow_low_precision("centered activations fp16"):
                TT(cfg["z_hi"]).tensor_tensor(
                    v3(z[:, :], 0, [[F, ncb], [NW, K], [1, NW]]),
                    v3(h_in[:, :], 0, [[F, ncb], [NW, K], [1, NW]]),
                    v3(mh[:, :], 0, [[NW, ncb], [0, K], [1, NW]]),
                    op=ALU.subtract)

            def write_half(ssq, half):
                sq_ssq(z, ncb, out_tile, "ssq_hi", tagp, ssq, half)
            return z, write_half

        def norm_psum(pzs, ncb, out_tile, tagp):
            """Norm+gelu for PSUM inputs.

            id9 form: DVE window-sums from PSUM, PE folds -mean (id9 matmul),
            ACT evacuates centered z.
            raw form: ACT evacuates raw y, window-sums via tree on SBUF,
            DVE subtracts the mean."""
            if cfg["n23"] == "id9":
                nsum = stat.tile([128, ncb * NW], RT, tag="ns", name=f"ns{tagp}", bufs=6)
                with nc.allow_low_precision("fp16 window sums drive mean only"):
                    for cb in range(ncb):
                        nc.vector.tensor_reduce(
                            nsum[:, cb * NW:(cb + 1) * NW],
                            v3(pzs[cb][:, :], 0, [[1, NW], [NW, K]]),
                            axis=AX.X, op=ALU.add, negate=True,
                        )
                z = work.tile([128, ncb * F], RT, tag="z23", name=f"z{tagp}", bufs=(3 if cfg["n23"] == "raw" else 4))
                for cb in range(ncb):
                    nsb = v3(nsum[:, :], cb * NW, [[0, K], [1, NW]])
                    nc.tensor.matmul(pzs[cb][:, :F], id9sb[:, :], nsb,
                                     start=False, stop=True, skip_group_check=True)
                    if cfg["evac_z"] == "act":
                        nc.scalar.copy(z[:, cb * F:(cb + 1) * F], pzs[cb][:, :F])
                    else:
                        with nc.allow_low_precision("z in fp16"):
                            nc.vector.tensor_copy(z[:, cb * F:(cb + 1) * F],
                                                  pzs[cb][:, :F])
            else:
                y0 = work.tile([128, ncb * F], RT, tag="y23", name=f"y{tagp}", bufs=3)
                for cb in range(ncb):
                    nc.scalar.copy(y0[:, cb * F:(cb + 1) * F], pzs[cb][:, :F])
                ns = stat.tile([128, ncb * NW], RT, tag="ns", name=f"ns{tagp}", bufs=6)
                how = cfg["ns_lo"]
                if how == "dred":
                    with nc.allow_low_precision("fp16 window sums"):
                        nc.vector.tensor_reduce(
                            v3(ns[:, :], 0, [[NW, ncb], [1, NW]]),
                            v3(y0[:, :], 0, [[F, ncb], [1, NW], [NW, K]]),
                            axis=AX.X, op=ALU.add,
                        )
                else:
                    tree9("dve" if how == "dtree" else "pool", y0[:, :], 0, ncb,
                          v3(ns[:, :], 0, [[NW, ncb], [1, NW]]), "m")
                mh = stat.tile([128, ncb * NW], RT, tag="mh", name=f"mh{tagp}", bufs=6)
                with nc.allow_low_precision("mean in fp16"):
                    nc.vector.tensor_scalar(mh[:, :], ns[:, :], 1.0 / 9.0, None,
                                            op0=ALU.mult)
                z = work.tile([128, ncb * F], RT, tag="z23", name=f"z{tagp}", bufs=(3 if cfg["n23"] == "raw" else 4))
                with nc.allow_low_precision("centered activations fp16"):
                    TT(cfg["z_lo"]).tensor_tensor(
                        v3(z[:, :], 0, [[F, ncb], [NW, K], [1, NW]]),
                        v3(y0[:, :], 0, [[F, ncb], [NW, K], [1, NW]]),
                        v3(mh[:, :], 0, [[NW, ncb], [0, K], [1, NW]]),
                        op=ALU.subtract)

            def write_half(ssq, half):
                sq_ssq(z, ncb, out_tile, "ssq_lo", tagp, ssq, half)
            return z, write_half

        # ---- software-pipelined wavefront over window tiles ----
        state = {}
        pend = {}  # site -> (ssq_tile, [(z, ncb, out_tile, qv_ap_or_None)])

        def norm_finish_site(key, ncb, tagp):
            ssq, ctxs = pend.pop(key)
            n = len(ctxs) * ncb * NW
            s16 = chain_rsqrt(ssq[:, :n], n, tagp)
            for half, (z, out_tile) in enumerate(ctxs):
                s = bass.AP(s16[:, :].tensor, s16[:, :].offset + half * ncb * NW,
                            [list(s16[:, :].ap[0]), [1, ncb * NW]])
                finish_norm(z, s, ncb, out_tile,
                            cfg["aff_hi" if ncb == NCB_HI else "aff_lo"])

        def norm_add_half(key, ncb, tagp, z, out_tile, solo, write_half):
            """write_half(ssq_tile, half) must fill cols [half*n, (half+1)*n)."""
            if cfg.get("pair", "on") == "off":
                solo = True
            if key not in pend:
                ssq = stat.tile([128, 2 * ncb * NW], RT, tag="ssq",
                                name=f"ssq{tagp}", bufs=4)
                pend[key] = (ssq, [])
            ssq, ctxs = pend[key]
            half = len(ctxs)
            write_half(ssq, half)
            ctxs.append((z, out_tile))
            if solo or half == 1:
                norm_finish_site(key, ncb, tagp)

        def x_tw_views(ti, li):
            w0 = ti * NW
            if li == 0:
                return [v3(eTall[:, :], cb * LSEQ + w0, [[1, K], [1, NW]])
                        for cb in range(NCB_HI)]
            h_in = state[ti]["h0"]
            return [v3(h_in[:, :], cb * F, [[NW, K], [1, NW]])
                    for cb in range(NCB_HI)]

        def s_norm1(ti, li):
            ga = work.tile([128, NCB_HI * F], RT, tag="ga", name="ga")
            if li == 0:
                z, wh = norm_l0(ti, ga)
                solo = ti - 6 < 0 or cfg.get("pair", "off") == "off"
                norm_add_half(("n1", ti), NCB_HI, "n1", z, ga, solo, wh)
            else:
                z, wh = norm_sbuf_l1(state[ti]["h0"], ga)
                solo = ti + 6 >= n_tiles
                norm_add_half(("n1", ti + 6), NCB_HI, "n1", z, ga, solo, wh)
            state[ti]["ga"] = ga

        def s_mm1(ti, li):
            ga = state[ti].pop("ga")
            pm1t = []
            for mb in range(NCB_LO):
                pm = psm1.tile([128, F], FP, tag="pm1", name="pm")
                for kb in range(NCB_HI):
                    nc.tensor.matmul(
                        pm[:, :F],
                        w1sb[li][:, kb * LOW + mb * 128: kb * LOW + mb * 128 + 128],
                        ga[:, kb * F:(kb + 1) * F],
                        start=(kb == 0),
                        stop=(cfg["n23"] == "raw" and kb == NCB_HI - 1),
                    )
                pm1t.append(pm)
            state[ti]["pm1t"] = pm1t

        def s_norm2(ti, li):
            pm1t = state[ti].pop("pm1t")
            gb = work.tile([128, NCB_LO * F], RT, tag="gb", name="gb")
            z, wh = norm_psum(pm1t, NCB_LO, gb, f"n2l{li}")
            # pair with norm3 of the same layer, tile ti-2 (same wavefront step)
            step = ti + (2 if li == 0 else 8)
            solo = ti - 2 < 0
            norm_add_half((f"lo{li}", step), NCB_LO, f"n2l{li}", z, gb, solo, wh)
            state[ti]["gb"] = gb

        def s_conv(ti, li):
            gb = state[ti].pop("gb")
            pcvt = []
            for mb in range(NCB_LO):
                pc = pscv.tile([128, F], FP, tag="pcv", name="pc")
                first = True
                for d in (0, -1, 1, -2, 2):
                    t0 = max(0, -d)
                    t1 = min(K, K - d)
                    n = t1 - t0
                    for kb in range(NCB_LO):
                        j = (d + 2) * NCB_LO + kb
                        nc.tensor.matmul(
                            v3(pc[:, :], t0 * NW, [[NW, n], [1, NW]]),
                            w2sb[li][:, j * LOW + mb * 128: j * LOW + mb * 128 + 128],
                            v3(gb[:, :], kb * F + (t0 + d) * NW, [[NW, n], [1, NW]]),
                            start=first,
                            stop=(cfg["n23"] == "raw" and d == 2 and kb == NCB_LO - 1),
                            skip_group_check=True,
                        )
                        first = False
                pcvt.append(pc)
            state[ti]["pcvt"] = pcvt

        def s_norm3(ti, li):
            pcvt = state[ti].pop("pcvt")
            gc = work.tile([128, NCB_LO * F], RT, tag="gc", name="gc")
            z, wh = norm_psum(pcvt, NCB_LO, gc, f"n3l{li}")
            step = ti + (4 if li == 0 else 10)
            solo = ti + 2 >= n_tiles
            norm_add_half((f"lo{li}", step), NCB_LO, f"n3l{li}", z, gc, solo, wh)
            state[ti]["gc"] = gc

        def s_mm3(ti, li):
            gc = state[ti].pop("gc")
            x_tw = x_tw_views(ti, li)
            h_out = work.tile([128, NCB_HI * F], RT, tag=f"h{li}", name=f"h{li}",
                              bufs=(8 if int(os.environ.get("SMODE", "13")) == 13 else 5) if li == 0 else 3)
            for cb in range(NCB_HI):
                pm = psm3.tile([128, F], FP, tag="pm3", name="pm3")
                for kb in range(NCB_LO):
                    nc.tensor.matmul(
                        pm[:, :F],
                        w3sb[li][:, kb * DIM + cb * 128: kb * DIM + cb * 128 + 128],
                        gc[:, kb * F:(kb + 1) * F],
                        start=(kb == 0), stop=False,
                    )
                nc.tensor.matmul(pm[:, :F], id1sb[:, :], x_tw[cb],
                                 start=False, stop=True, skip_group_check=True)
                if cfg["evac_h"] == "act":
                    nc.scalar.copy(h_out[:, cb * F:(cb + 1) * F], pm[:, :F])
                else:
                    with nc.allow_low_precision("h in fp16"):
                        nc.vector.tensor_copy(h_out[:, cb * F:(cb + 1) * F],
                                              pm[:, :F])
            if li == 1:
                state[ti].pop("h0", None)
            state[ti][f"h{li}"] = h_out

        def s_outproj(ti, li):
            w0 = ti * NW
            h_in = state[ti].pop("h1")
            po = pso.tile([VOCAB, NW], FP, tag="po", name="po")
            first = True
            for cb in range(NCB_HI):
                for t in range(K):
                    j = cb * K + t
                    nc.tensor.matmul(
                        po[:, :],
                        owsb[:, j * VOCAB:(j + 1) * VOCAB],
                        h_in[:, cb * F + t * NW: cb * F + t * NW + NW],
                        start=first, stop=(j == NCB_HI * K - 1),
                    )
                    first = False
            with nc.allow_low_precision("logits in fp16"):
                nc.scalar.copy(osb[:, w0:w0 + NW], po[:, :])
            del state[ti]

        def merge(*fns):
            def g(ti, li):
                for f in fns:
                    f(ti, li)
            return g

        SMODE = int(os.environ.get("SMODE", "13"))
        stages = []
        if SMODE == 13:
            for li in range(NL):
                stages += [(s_norm1, li), (s_mm1, li), (s_norm2, li),
                           (s_conv, li), (s_norm3, li), (s_mm3, li)]
            stages.append((s_outproj, None))
        elif SMODE == 7:
            for li in range(NL):
                stages += [(merge(s_norm1, s_mm1), li),
                           (merge(s_norm2, s_conv), li),
                           (merge(s_norm3, s_mm3), li)]
            stages.append((s_outproj, None))
        else:  # 5
            for li in range(NL):
                stages += [(merge(s_norm1, s_mm1, s_norm2), li),
                           (merge(s_conv, s_norm3, s_mm3), li)]
            stages.append((s_outproj, None))
        n_stages = len(stages)

        order = os.environ.get("SORDER", "new")
        for step in range(n_tiles + n_stages - 1):
            sis = range(n_stages) if order == "new" else range(n_stages - 1, -1, -1)
            for si in sis:
                ti = step - si
                if 0 <= ti < n_tiles:
                    if si == 0:
                        state[ti] = {}
                    fn, li = stages[si]
                    fn(ti, li)

        nc.sync.dma_start(out_d[:, :], osb[:, :])

    nc.compile()
    return nc


_CACHE = {}


def _get_nc(n_tiles, cfg=None):
    key = (n_tiles, tuple(sorted((cfg or CFG).items())))
    if key not in _CACHE:
        _CACHE[key] = build(n_tiles, cfg)
    return _CACHE[key]


def _prep_inputs(x, emb, ln1_w, ln1_b, ln2_w, ln2_b, ln3_w, ln3_b,
                 c1_w, c1_b, c2_w, c2_b, c3_w, c3_b, out_w, out_b):
    f32 = lambda a: np.ascontiguousarray(np.asarray(a), dtype=np.float32)
    rt = lambda a: np.ascontiguousarray(np.asarray(a, dtype=np.float32), dtype=NPRT)
    x = np.asarray(x)
    oneh = (x[:, None, :] == np.arange(VOCAB)[None, :, None]).astype(NPRT)

    c1_w, c2_w, c3_w = f32(c1_w), f32(c2_w), f32(c3_w)
    assert np.all(np.asarray(c1_b) == 0) and np.all(np.asarray(c2_b) == 0) \
        and np.all(np.asarray(c3_b) == 0), "conv biases assumed zero"
    for g in (ln1_w, ln2_w, ln3_w):
        assert np.all(np.asarray(g) == 1), "instance-norm gamma assumed one"
    for b in (ln1_b, ln2_b, ln3_b):
        assert np.all(np.asarray(b) == 0), "instance-norm beta assumed zero"
    assert np.all(np.asarray(out_b) == 0), "output bias assumed zero"

    w1h = rt(c1_w.transpose(0, 2, 1).reshape(NL, NCB_HI, 128, LOW))
    w2h = rt(c2_w.transpose(0, 3, 2, 1).reshape(NL, 5, NCB_LO, 128, LOW))
    w3h = rt(c3_w.transpose(0, 2, 1).reshape(NL, NCB_LO, 128, DIM))
    owh = rt(f32(out_w).reshape(VOCAB, NCB_HI, 128, K).transpose(1, 3, 2, 0))

    shared = {
        "embw": rt(emb), "w1": w1h, "w2": w2h, "w3": w3h, "ow": owh,
        "id1": np.eye(128, dtype=NPRT),
        "id9": (np.eye(128, dtype=np.float32) / 9.0).astype(NPRT),
    }
    in_maps = [{"oneh": np.ascontiguousarray(oneh[b]), **shared} for b in range(B)]
    return in_maps


def run(inputs, n_tiles=NT, n_cores=B, trace=False):
    nc = _get_nc(n_tiles)
    in_maps = _prep_inputs(**inputs)[:n_cores]
    res = run_bass_kernel_spmd(nc, in_maps, core_ids=list(range(n_cores)), trace=trace)
    out = np.stack([res.results[i]["out"].astype(np.float32).T
                    for i in range(n_cores)])
    return out, res


def kernel(**inputs):
    out, _ = run(inputs)
    return out.astype(np.float32)


# revision 4
# speedup vs baseline: 1.0049x; 1.0049x over previous
"""Trainium2 Bass kernel for a ByteNet-style Markov LM over sliding windows.

x (8, 2048) int tokens -> emb windows (B*W, 512, 9) -> 2 ByteNet layers
(inorm+gelu, 1x1 512->256, inorm+gelu, k=5 conv 256->256, inorm+gelu,
1x1 256->512, residual) -> flatten -> (4608 -> 7) projection -> (8, 2040, 7).

Sharding: pure data parallel, one batch row per NeuronCore (8 cores).

v3 design:
  - Software-pipelined wavefront emission (13 stages, ~13 tiles in flight)
    so every engine's FIFO interleaves independent tiles.
  - gamma=1/beta=0/out_b=0 asserted on host and dropped from the kernel.
  - Layer-0 norm1 stats via global sliding 9-sums S (and S2 of squares)
    precomputed once over the position axis (windows overlap; per-tile cost
    is a handful of small ops instead of big reduces).
  - fp16 packed-SBUF elementwise (DVE 2x modes); per-op-site engine knobs
    (DVE / Pool / ACT-Identity) for load balancing, tuned via the cost sim.
  - rsqrt via fp32 magic seed + 1 Newton step (max 0.18% rel err).
  - sq scratch written into the norm's output tile (dead until affine).
  - Output projection transposed: ow stationary (7 cols), h moving;
    (7, 51) PSUM tiles -> (7, 2040) fp16 SBUF strip -> one DMA -> host
    transpose.
"""

import os
from contextlib import ExitStack

import numpy as np

os.environ.setdefault("MYCRO_LOCAL_CACHE", "1")

import concourse.bass as bass
import concourse.bacc as bacc
import concourse.mybir as mybir
from concourse import tile
from concourse.bass_utils import run_bass_kernel_spmd

FP = mybir.dt.float32
U32 = mybir.dt.uint32
RT = mybir.dt.float16
NPRT = np.float16
AF = mybir.ActivationFunctionType
ALU = mybir.AluOpType
AX = mybir.AxisListType

K = 9
VOCAB = 7
DIM = 512
LOW = 256
LSEQ = 2048
B = 8
W = LSEQ - K + 1  # 2040
NW = 51           # windows per tile
NT = W // NW      # 40 tiles
F = K * NW        # 459 free elements per (cb, tile)
NL = 2
EPS = 1e-5
MAGIC = float(0x5f3759df)

NCB_HI = DIM // 128   # 4
NCB_LO = LOW // 128   # 2

# Engine assignment knobs (tuned against the cost-model sim).
CFG = {
    "sq_hi": "dve",
    "sq_lo": "dve",
    "ssq_hi": "ptree",
    "ssq_lo": "dtree",
    "ns_hi": "ptree",
    "aff_hi": "dve",
    "aff_lo": "dve",
    "z_hi": "dve",
    "z_lo": "dve",
    "qv": "pool",
    "wv": "act",
    "y0f": "pool",
    "ysq": "act",
    "yc": "act",     # fp->int convert: act|dve|pool
    "n23": "id9",    # norm2/3 mean handling: id9|raw
    "ns_lo": "ptree",  # raw-form window sums: dred|dtree|ptree
    "tree1": "dve",  # tree stage-1 engine: dve|pool
    "u": "dve",      # newton u = ysq*qv: dve|pool
    "s16": "dve",    # s16 = y*wv: dve|pool
    "evac_h": "act",  # mm3 PSUM evac: act|dve
    "evac_z": "act",  # norm2/3 z evac: act|dve
    "pair": "off",   # cross-layer chain pairing
}


def v3(base, off, dims):
    """View of a 2D (P, F) AP with explicit free dims [[step, count], ...]."""
    return bass.AP(base.tensor, base.offset + off, [list(base.ap[0])] + [list(d) for d in dims])


def build(n_tiles=NT, cfg=None):
    cfg = dict(CFG if cfg is None else cfg)
    nc = bacc.Bacc("TRN2", target_bir_lowering=False, debug=False)

    # ---- DRAM I/O ----
    oneh_d = nc.dram_tensor("oneh", [VOCAB, LSEQ], RT, kind="ExternalInput")
    emb_d = nc.dram_tensor("embw", [VOCAB, DIM], RT, kind="ExternalInput")
    w1_d = nc.dram_tensor("w1", [NL, NCB_HI, 128, LOW], RT, kind="ExternalInput")
    w2_d = nc.dram_tensor("w2", [NL, 5, NCB_LO, 128, LOW], RT, kind="ExternalInput")
    w3_d = nc.dram_tensor("w3", [NL, NCB_LO, 128, DIM], RT, kind="ExternalInput")
    ow_d = nc.dram_tensor("ow", [NCB_HI, K, 128, VOCAB], RT, kind="ExternalInput")
    id1_d = nc.dram_tensor("id1", [128, 128], RT, kind="ExternalInput")
    id9_d = nc.dram_tensor("id9", [128, 128], RT, kind="ExternalInput")
    out_d = nc.dram_tensor("out", [VOCAB, W], RT, kind="ExternalOutput")

    with tile.TileContext(nc) as tc, ExitStack() as ctx:
        const = ctx.enter_context(tc.tile_pool(name="const", bufs=1))
        work = ctx.enter_context(tc.tile_pool(name="work", bufs=3))
        stat = ctx.enter_context(tc.tile_pool(name="stat", bufs=4))
        rings = cfg.get("rings", (2, 2, 2, 2))
        psm1 = ctx.enter_context(tc.tile_pool(name="psm1", bufs=rings[0], space="PSUM"))
        pscv = ctx.enter_context(tc.tile_pool(name="pscv", bufs=rings[1], space="PSUM"))
        psm3 = ctx.enter_context(tc.tile_pool(name="psm3", bufs=rings[2], space="PSUM"))
        pso = ctx.enter_context(tc.tile_pool(name="pso", bufs=rings[3], space="PSUM"))

        def TT(e):
            return nc.vector if e == "dve" else nc.gpsimd

        # ---- load constants ----
        onehsb = const.tile([VOCAB, LSEQ], RT)
        nc.sync.dma_start(onehsb[:, :], oneh_d[:, :])
        embsb = const.tile([VOCAB, DIM], RT)
        nc.sync.dma_start(embsb[:, :], emb_d[:, :])

        w1sb, w2sb, w3sb = [], [], []
        for i in range(NL):
            t1 = const.tile([128, NCB_HI * LOW], RT, name=f"w1sb{i}")
            for kb in range(NCB_HI):
                nc.sync.dma_start(t1[:, kb * LOW:(kb + 1) * LOW], w1_d[i, kb])
            w1sb.append(t1)
            t2 = const.tile([128, 5 * NCB_LO * LOW], RT, name=f"w2sb{i}")
            for d in range(5):
                for kb in range(NCB_LO):
                    j = d * NCB_LO + kb
                    nc.sync.dma_start(t2[:, j * LOW:(j + 1) * LOW], w2_d[i, d, kb])
            w2sb.append(t2)
            t3 = const.tile([128, NCB_LO * DIM], RT, name=f"w3sb{i}")
            for kb in range(NCB_LO):
                nc.sync.dma_start(t3[:, kb * DIM:(kb + 1) * DIM], w3_d[i, kb])
            w3sb.append(t3)

        owsb = const.tile([128, NCB_HI * K * VOCAB], RT)
        for cb in range(NCB_HI):
            for t in range(K):
                j = cb * K + t
                nc.sync.dma_start(owsb[:, j * VOCAB:(j + 1) * VOCAB], ow_d[cb, t])

        id1sb = const.tile([128, 128], RT)
        nc.sync.dma_start(id1sb[:, :], id1_d[:, :])
        id9sb = const.tile([128, 128], RT)
        nc.sync.dma_start(id9sb[:, :], id9_d[:, :])

        # output strip (7 partitions x W) in fp16
        osb = const.tile([VOCAB, W], RT)

        epsb = const.tile([128, 1], FP)
        nc.gpsimd.memset(epsb[:, :], float(EPS))
        c15b = const.tile([128, 1], FP)
        nc.gpsimd.memset(c15b[:, :], 1.5)

        # ---- embedding: eTall (128, 4*LSEQ); block cb at cols [cb*LSEQ, ...) ----
        eTall = const.tile([128, NCB_HI * LSEQ], RT)
        for cb in range(NCB_HI):
            for ch in range(LSEQ // 512):
                pe_ps = psm1.tile([128, 512], FP, tag="pm1", name="pe_ps")
                nc.tensor.matmul(
                    pe_ps[:, :],
                    embsb[:, cb * 128:(cb + 1) * 128],
                    onehsb[:, ch * 512:(ch + 1) * 512],
                    start=True, stop=True,
                )
                nc.scalar.copy(eTall[:, cb * LSEQ + ch * 512: cb * LSEQ + ch * 512 + 512],
                               pe_ps[:, :])

        # ---- global sliding 9-sums over positions: S (sums), S2 (sums of sq) ----
        # S[cb, p] = sum_{t<9} eT[cb, p+t], valid p in [0, W).
        Ssb = const.tile([128, NCB_HI * LSEQ], RT)
        S2sb = const.tile([128, NCB_HI * LSEQ], RT)
        with nc.allow_low_precision("fp16 sliding sums drive mean/var"):
            for cb in range(NCB_HI):
                o = cb * LSEQ
                e2c = work.tile([128, LSEQ], RT, tag="pre_e", name="e2c", bufs=1)
                nc.vector.tensor_mul(e2c[:, :], eTall[:, o:o + LSEQ], eTall[:, o:o + LSEQ])
                for (src, srco, dst) in ((eTall, o, Ssb), (e2c, 0, S2sb)):
                    ac = work.tile([128, LSEQ], RT, tag="pre_a", name="ac", bufs=1)
                    n3 = LSEQ - 6
                    nc.vector.tensor_tensor(
                        ac[:, :n3], v3(src[:, :], srco, [[1, n3]]),
                        v3(src[:, :], srco + 3, [[1, n3]]), op=ALU.add)
                    nc.gpsimd.tensor_tensor(
                        ac[:, :n3], ac[:, :n3],
                        v3(src[:, :], srco + 6, [[1, n3]]), op=ALU.add)
                    nc.vector.tensor_tensor(
                        dst[:, o:o + W], ac[:, :W], ac[:, 1:W + 1], op=ALU.add)
                    nc.gpsimd.tensor_tensor(
                        dst[:, o:o + W], dst[:, o:o + W], ac[:, 2:W + 2], op=ALU.add)

        # ---- helpers ----
        def tree9(e, src_base, src_off, ncb, out_ap, tag):
            """out (128, ncb*NW) fp16 = per-window sums over t of a contiguous
            (128, ncb*F) fp16 region via packed tensor-adds (engine e)."""
            a = work.tile([128, ncb * 3 * NW], RT, tag=f"tr{tag}", name=f"tr{tag}", bufs=3)
            TN = 3 * NW
            e1 = cfg["tree1"]
            av = v3(a[:, :], 0, [[TN, ncb], [1, TN]])
            with nc.allow_low_precision("fp16 window sums"):
                TT(e1).tensor_tensor(
                    av, v3(src_base, src_off, [[F, ncb], [1, TN]]),
                    v3(src_base, src_off + TN, [[F, ncb], [1, TN]]), op=ALU.add)
                TT(e1).tensor_tensor(
                    av, av, v3(src_base, src_off + 2 * TN, [[F, ncb], [1, TN]]),
                    op=ALU.add)
                TT(e).tensor_tensor(
                    out_ap, v3(a[:, :], 0, [[TN, ncb], [1, NW]]),
                    v3(a[:, :], NW, [[TN, ncb], [1, NW]]), op=ALU.add)
                TT(e).tensor_tensor(
                    out_ap, out_ap, v3(a[:, :], 2 * NW, [[TN, ncb], [1, NW]]),
                    op=ALU.add)

        def chain_rsqrt(ssq_ap, n, tagp):
            """s16 (128, n) fp16 = rsqrt(ssq/9 + eps), magic + 1 Newton."""
            qvt = stat.tile([128, n], FP, tag="qv", name=f"qv{tagp}")
            if cfg["qv"] == "act":
                nc.scalar.activation(qvt[:, :], ssq_ap, AF.Identity,
                                     bias=epsb[:, :], scale=1.0 / 9.0)
            elif cfg["qv"] == "dve":
                nc.vector.tensor_scalar(qvt[:, :], ssq_ap, 1.0 / 9.0, float(EPS),
                                        op0=ALU.mult, op1=ALU.add)
            else:
                nc.gpsimd.tensor_scalar(qvt[:, :], ssq_ap, 1.0 / 9.0, float(EPS),
                                        op0=ALU.mult, op1=ALU.add)
            qv = qvt[:, :]
            y0f = stat.tile([128, n], FP, tag="y0f", name=f"y0f{tagp}")
            if cfg["y0f"] == "dve":
                nc.vector.tensor_scalar(y0f[:, :], qv.bitcast(U32), -0.5,
                                        MAGIC, op0=ALU.mult, op1=ALU.add)
            else:
                nc.gpsimd.tensor_scalar(y0f[:, :], qv.bitcast(U32), -0.5,
                                        MAGIC, op0=ALU.mult, op1=ALU.add)
            yc = stat.tile([128, n], FP, tag="yc", name=f"yc{tagp}")
            if cfg["yc"] == "act":
                nc.scalar.copy(yc[:, :].bitcast(U32), y0f[:, :])  # fp value -> bits
            elif cfg["yc"] == "dve":
                nc.vector.tensor_copy(yc[:, :].bitcast(U32), y0f[:, :])
            else:
                nc.gpsimd.tensor_copy(yc[:, :].bitcast(U32), y0f[:, :])
            y = yc[:, :].bitcast(FP)
            ysq = stat.tile([128, n], FP, tag="ysq", name=f"ysq{tagp}")
            if cfg["ysq"] == "act":
                nc.scalar.activation(ysq[:, :], y, AF.Square)
            else:
                nc.vector.tensor_mul(ysq[:, :], y, y)
            # u = ysq*qv; wv = 1.5 - 0.5*u  (in-place on ysq)
            TT(cfg["u"]).tensor_mul(ysq[:, :], ysq[:, :], qv)
            if cfg["wv"] == "act":
                nc.scalar.activation(ysq[:, :], ysq[:, :], AF.Identity,
                                     bias=c15b[:, :], scale=-0.5)
            elif cfg["wv"] == "dve":
                nc.vector.tensor_scalar(ysq[:, :], ysq[:, :], -0.5, 1.5,
                                        op0=ALU.mult, op1=ALU.add)
            else:
                nc.gpsimd.tensor_scalar(ysq[:, :], ysq[:, :], -0.5, 1.5,
                                        op0=ALU.mult, op1=ALU.add)
            s16 = stat.tile([128, n], RT, tag="s16", name=f"s16{tagp}")
            with nc.allow_low_precision("norm scale in fp16"):
                TT(cfg["s16"]).tensor_mul(s16[:, :], y, ysq[:, :])
            return s16

        def sq_ssq(z, ncb, scratch, kind, tagp, ssq, half):
            """Write per-window sums of z^2 into half of the paired ssq tile."""
            e_sq = cfg["sq_hi" if ncb == NCB_HI else "sq_lo"]
            sqv = scratch[:, :ncb * F]
            zf = z[:, :ncb * F]
            with nc.allow_low_precision("z^2 in fp16"):
                TT(e_sq).tensor_mul(sqv, zf, zf)
            off = half * ncb * NW
            how = cfg[kind]
            if how == "dred":
                with nc.allow_low_precision("fp16 ssq"):
                    nc.vector.tensor_reduce(
                        v3(ssq[:, :], off, [[NW, ncb], [1, NW]]),
                        v3(scratch[:, :], 0, [[F, ncb], [1, NW], [NW, K]]),
                        axis=AX.X, op=ALU.add,
                    )
            else:
                tree9("dve" if how == "dtree" else "pool", scratch[:, :], 0, ncb,
                      v3(ssq[:, :], off, [[NW, ncb], [1, NW]]), "s")

        def finish_norm(z, s16, ncb, out_tile, e_aff):
            """out = gelu(z * bcast(s16)); z fp16 (128, ncb*F) SBUF."""
            zv = v3(z[:, :], 0, [[F, ncb], [NW, K], [1, NW]])
            ov = v3(out_tile[:, :], 0, [[F, ncb], [NW, K], [1, NW]])
            sb = v3(s16[:, :], 0, [[NW, ncb], [0, K], [1, NW]])
            with nc.allow_low_precision("normalized activations in fp16"):
                TT(e_aff).tensor_mul(ov, zv, sb)
            nc.scalar.activation(out_tile[:, :ncb * F], out_tile[:, :ncb * F],
                                 AF.Gelu)

        def norm_l0(ti, out_tile):
            """Layer-0 norm1: stats from the global sliding sums S/S2."""
            w0 = ti * NW
            ncb = NCB_HI
            n = ncb * NW
            tagp = "n1l0"
            mh = stat.tile([128, n], RT, tag="mh", name="mh0", bufs=6)
            with nc.allow_low_precision("mean in fp16"):
                nc.vector.tensor_scalar(
                    mh[:, :], v3(Ssb[:, :], w0, [[LSEQ, ncb], [1, NW]]),
                    1.0 / 9.0, None, op0=ALU.mult)
            # z = x - bcast(mh); (cb, t, w) layout matches the F flat order
            z = work.tile([128, ncb * F], RT, tag="z1", name="z1")
            with nc.allow_low_precision("centered activations fp16"):
                TT(cfg["z_hi"]).tensor_tensor(
                    v3(z[:, :], 0, [[F, ncb], [NW, K], [1, NW]]),
                    v3(eTall[:, :], w0, [[LSEQ, ncb], [1, K], [1, NW]]),
                    v3(mh[:, :], 0, [[NW, ncb], [0, K], [1, NW]]),
                    op=ALU.subtract)

            def write_half(ssq, half):
                # ssq half = Sigma z^2 = S2 - 9*m^2 (fp16 smalls)
                off = half * n
                m2 = stat.tile([128, n], RT, tag="mh", name="m2l0", bufs=6)
                with nc.allow_low_precision("m^2 in fp16"):
                    nc.vector.tensor_mul(m2[:, :], mh[:, :], mh[:, :])
                    nc.vector.tensor_scalar(m2[:, :], m2[:, :], -9.0, None,
                                            op0=ALU.mult)
                    nc.vector.tensor_tensor(
                        v3(ssq[:, :], off, [[1, n]]),
                        v3(S2sb[:, :], w0, [[LSEQ, ncb], [1, NW]]),
                        m2[:, :], op=ALU.add)
            return z, write_half

        def norm_sbuf_l1(h_in, out_tile):
            """Layer-1 norm1 on SBUF fp16 h (128, ncb_hi*F)."""
            ncb = NCB_HI
            tagp = "n1l1"
            ns = stat.tile([128, ncb * NW], RT, tag="ns", name="ns1", bufs=6)
            how = cfg["ns_hi"]
            if how == "dred":
                with nc.allow_low_precision("fp16 window sums"):
                    nc.vector.tensor_reduce(
                        v3(ns[:, :], 0, [[NW, ncb], [1, NW]]),
                        v3(h_in[:, :], 0, [[F, ncb], [1, NW], [NW, K]]),
                        axis=AX.X, op=ALU.add,
                    )
            else:
                tree9("dve" if how == "dtree" else "pool", h_in[:, :], 0, ncb,
                      v3(ns[:, :], 0, [[NW, ncb], [1, NW]]), "n")
            mh = stat.tile([128, ncb * NW], RT, tag="mh", name="mh1", bufs=6)
            with nc.allow_low_precision("mean in fp16"):
                nc.vector.tensor_scalar(mh[:, :], ns[:, :], 1.0 / 9.0, None,
                                        op0=ALU.mult)
            z = work.tile([128, ncb * F], RT, tag="z1", name="z1b")
            with nc.allow_low_precision("centered activations fp16"):
                TT(cfg["z_hi"]).tensor_tensor(
                    v3(z[:, :], 0, [[F, ncb], [NW, K], [1, NW]]),
                    v3(h_in[:, :], 0, [[F, ncb], [NW, K], [1, NW]]),
                    v3(mh[:, :], 0, [[NW, ncb], [0, K], [1, NW]]),
                    op=ALU.subtract)

            def write_half(ssq, half):
                sq_ssq(z, ncb, out_tile, "ssq_hi", tagp, ssq, half)
            return z, write_half

        def norm_psum(pzs, ncb, out_tile, tagp):
            """Norm+gelu for PSUM inputs.

            id9 form: DVE window-sums from PSUM, PE folds -mean (id9 matmul),
            ACT evacuates centered z.
            raw form: ACT evacuates raw y, window-sums via tree on SBUF,
            DVE subtracts the mean."""
            if cfg["n23"] == "id9":
                nsum = stat.tile([128, ncb * NW], RT, tag="ns", name=f"ns{tagp}", bufs=6)
                with nc.allow_low_precision("fp16 window sums drive mean only"):
                    for cb in range(ncb):
                        nc.vector.tensor_reduce(
                            nsum[:, cb * NW:(cb + 1) * NW],
                            v3(pzs[cb][:, :], 0, [[1, NW], [NW, K]]),
                            axis=AX.X, op=ALU.add, negate=True,
                        )
                z = work.tile([128, ncb * F], RT, tag="z23", name=f"z{tagp}", bufs=(3 if cfg["n23"] == "raw" else 4))
                for cb in range(ncb):
                    nsb = v3(nsum[:, :], cb * NW, [[0, K], [1, NW]])
                    nc.tensor.matmul(pzs[cb][:, :F], id9sb[:, :], nsb,
                                     start=False, stop=True, skip_group_check=True)
                    if cfg["evac_z"] == "act":
                        nc.scalar.copy(z[:, cb * F:(cb + 1) * F], pzs[cb][:, :F])
                    else:
                        with nc.allow_low_precision("z in fp16"):
                            nc.vector.tensor_copy(z[:, cb * F:(cb + 1) * F],
                                                  pzs[cb][:, :F])
            else:
                y0 = work.tile([128, ncb * F], RT, tag="y23", name=f"y{tagp}", bufs=3)
                for cb in range(ncb):
                    nc.scalar.copy(y0[:, cb * F:(cb + 1) * F], pzs[cb][:, :F])
                ns = stat.tile([128, ncb * NW], RT, tag="ns", name=f"ns{tagp}", bufs=6)
                how = cfg["ns_lo"]
                if how == "dred":
                    with nc.allow_low_precision("fp16 window sums"):
                        nc.vector.tensor_reduce(
                            v3(ns[:, :], 0, [[NW, ncb], [1, NW]]),
                            v3(y0[:, :], 0, [[F, ncb], [1, NW], [NW, K]]),
                            axis=AX.X, op=ALU.add,
                        )
                else:
                    tree9("dve" if how == "dtree" else "pool", y0[:, :], 0, ncb,
                          v3(ns[:, :], 0, [[NW, ncb], [1, NW]]), "m")
                mh = stat.tile([128, ncb * NW], RT, tag="mh", name=f"mh{tagp}", bufs=6)
                with nc.allow_low_precision("mean in fp16"):
                    nc.vector.tensor_scalar(mh[:, :], ns[:, :], 1.0 / 9.0, None,
                                            op0=ALU.mult)
                z = work.tile([128, ncb * F], RT, tag="z23", name=f"z{tagp}", bufs=(3 if cfg["n23"] == "raw" else 4))
                with nc.allow_low_precision("centered activations fp16"):
                    TT(cfg["z_lo"]).tensor_tensor(
                        v3(z[:, :], 0, [[F, ncb], [NW, K], [1, NW]]),
                        v3(y0[:, :], 0, [[F, ncb], [NW, K], [1, NW]]),
                        v3(mh[:, :], 0, [[NW, ncb], [0, K], [1, NW]]),
                        op=ALU.subtract)

            def write_half(ssq, half):
                sq_ssq(z, ncb, out_tile, "ssq_lo", tagp, ssq, half)
            return z, write_half

        # ---- software-pipelined wavefront over window tiles ----
        state = {}
        pend = {}  # site -> (ssq_tile, [(z, ncb, out_tile, qv_ap_or_None)])

        def norm_finish_site(key, ncb, tagp):
            ssq, ctxs = pend.pop(key)
            n = len(ctxs) * ncb * NW
            s16 = chain_rsqrt(ssq[:, :n], n, tagp)
            for half, (z, out_tile) in enumerate(ctxs):
                s = bass.AP(s16[:, :].tensor, s16[:, :].offset + half * ncb * NW,
                            [list(s16[:, :].ap[0]), [1, ncb * NW]])
                finish_norm(z, s, ncb, out_tile,
                            cfg["aff_hi" if ncb == NCB_HI else "aff_lo"])

        def norm_add_half(key, ncb, tagp, z, out_tile, solo, write_half):
            """write_half(ssq_tile, half) must fill cols [half*n, (half+1)*n)."""
            if cfg.get("pair", "on") == "off":
                solo = True
            if key not in pend:
                ssq = stat.tile([128, 2 * ncb * NW], RT, tag="ssq",
                                name=f"ssq{tagp}", bufs=4)
                pend[key] = (ssq, [])
            ssq, ctxs = pend[key]
            half = len(ctxs)
            write_half(ssq, half)
            ctxs.append((z, out_tile))
            if solo or half == 1:
                norm_finish_site(key, ncb, tagp)

        def x_tw_views(ti, li):
            w0 = ti * NW
            if li == 0:
                return [v3(eTall[:, :], cb * LSEQ + w0, [[1, K], [1, NW]])
                        for cb in range(NCB_HI)]
            h_in = state[ti]["h0"]
            return [v3(h_in[:, :], cb * F, [[NW, K], [1, NW]])
                    for cb in range(NCB_HI)]

        def s_norm1(ti, li):
            ga = work.tile([128, NCB_HI * F], RT, tag="ga", name="ga")
            if li == 0:
                z, wh = norm_l0(ti, ga)
                solo = ti - 6 < 0 or cfg.get("pair", "off") == "off"
                norm_add_half(("n1", ti), NCB_HI, "n1", z, ga, solo, wh)
            else:
                z, wh = norm_sbuf_l1(state[ti]["h0"], ga)
                solo = ti + 6 >= n_tiles
                norm_add_half(("n1", ti + 6), NCB_HI, "n1", z, ga, solo, wh)
            state[ti]["ga"] = ga

        def s_mm1(ti, li):
            ga = state[ti].pop("ga")
            pm1t = []
            for mb in range(NCB_LO):
                pm = psm1.tile([128, F], FP, tag="pm1", name="pm")
                for kb in range(NCB_HI):
                    nc.tensor.matmul(
                        pm[:, :F],
                        w1sb[li][:, kb * LOW + mb * 128: kb * LOW + mb * 128 + 128],
                        ga[:, kb * F:(kb + 1) * F],
                        start=(kb == 0),
                        stop=(cfg["n23"] == "raw" and kb == NCB_HI - 1),
                    )
                pm1t.append(pm)
            state[ti]["pm1t"] = pm1t

        def s_norm2(ti, li):
            pm1t = state[ti].pop("pm1t")
            gb = work.tile([128, NCB_LO * F], RT, tag="gb", name="gb")
            z, wh = norm_psum(pm1t, NCB_LO, gb, f"n2l{li}")
            # pair with norm3 of the same layer, tile ti-2 (same wavefront step)
            step = ti + (2 if li == 0 else 8)
            solo = ti - 2 < 0
            norm_add_half((f"lo{li}", step), NCB_LO, f"n2l{li}", z, gb, solo, wh)
            state[ti]["gb"] = gb

        def s_conv(ti, li):
            gb = state[ti].pop("gb")
            pcvt = []
            for mb in range(NCB_LO):
                pc = pscv.tile([128, F], FP, tag="pcv", name="pc")
                first = True
                for d in (0, -1, 1, -2, 2):
                    t0 = max(0, -d)
                    t1 = min(K, K - d)
                    n = t1 - t0
                    for kb in range(NCB_LO):
                        j = (d + 2) * NCB_LO + kb
                        nc.tensor.matmul(
                            v3(pc[:, :], t0 * NW, [[NW, n], [1, NW]]),
                            w2sb[li][:, j * LOW + mb * 128: j * LOW + mb * 128 + 128],
                            v3(gb[:, :], kb * F + (t0 + d) * NW, [[NW, n], [1, NW]]),
                            start=first,
                            stop=(cfg["n23"] == "raw" and d == 2 and kb == NCB_LO - 1),
                            skip_group_check=True,
                        )
                        first = False
                pcvt.append(pc)
            state[ti]["pcvt"] = pcvt

        def s_norm3(ti, li):
            pcvt = state[ti].pop("pcvt")
            gc = work.tile([128, NCB_LO * F], RT, tag="gc", name="gc")
            z, wh = norm_psum(pcvt, NCB_LO, gc, f"n3l{li}")
            step = ti + (4 if li == 0 else 10)
            solo = ti + 2 >= n_tiles
            norm_add_half((f"lo{li}", step), NCB_LO, f"n3l{li}", z, gc, solo, wh)
            state[ti]["gc"] = gc

        def s_mm3(ti, li):
            gc = state[ti].pop("gc")
            x_tw = x_tw_views(ti, li)
            h_out = work.tile([128, NCB_HI * F], RT, tag=f"h{li}", name=f"h{li}",
                              bufs=8 if li == 0 else 3)
            for cb in range(NCB_HI):
                pm = psm3.tile([128, F], FP, tag="pm3", name="pm3")
                for kb in range(NCB_LO):
                    nc.tensor.matmul(
                        pm[:, :F],
                        w3sb[li][:, kb * DIM + cb * 128: kb * DIM + cb * 128 + 128],
                        gc[:, kb * F:(kb + 1) * F],
                        start=(kb == 0), stop=False,
                    )
                nc.tensor.matmul(pm[:, :F], id1sb[:, :], x_tw[cb],
                                 start=False, stop=True, skip_group_check=True)
                if cfg["evac_h"] == "act":
                    nc.scalar.copy(h_out[:, cb * F:(cb + 1) * F], pm[:, :F])
                else:
                    with nc.allow_low_precision("h in fp16"):
                        nc.vector.tensor_copy(h_out[:, cb * F:(cb + 1) * F],
                                              pm[:, :F])
            if li == 1:
                state[ti].pop("h0", None)
            state[ti][f"h{li}"] = h_out

        def s_outproj(ti, li):
            w0 = ti * NW
            h_in = state[ti].pop("h1")
            po = pso.tile([VOCAB, NW], FP, tag="po", name="po")
            first = True
            for cb in range(NCB_HI):
                for t in range(K):
                    j = cb * K + t
                    nc.tensor.matmul(
                        po[:, :],
                        owsb[:, j * VOCAB:(j + 1) * VOCAB],
                        h_in[:, cb * F + t * NW: cb * F + t * NW + NW],
                        start=first, stop=(j == NCB_HI * K - 1),
                    )
                    first = False
            with nc.allow_low_precision("logits in fp16"):
                nc.scalar.copy(osb[:, w0:w0 + NW], po[:, :])
            del state[ti]

        def merge(*fns):
            def g(ti, li):
                for f in fns:
                    f(ti, li)
            return g

        SMODE = 13
        stages = []
        if SMODE == 13:
            for li in range(NL):
                stages += [(s_norm1, li), (s_mm1, li), (s_norm2, li),
                           (s_conv, li), (s_norm3, li), (s_mm3, li)]
            stages.append((s_outproj, None))
        elif SMODE == 7:
            for li in range(NL):
                stages += [(merge(s_norm1, s_mm1), li),
                           (merge(s_norm2, s_conv), li),
                           (merge(s_norm3, s_mm3), li)]
            stages.append((s_outproj, None))
        else:  # 5
            for li in range(NL):
                stages += [(merge(s_norm1, s_mm1, s_norm2), li),
                           (merge(s_conv, s_norm3, s_mm3), li)]
            stages.append((s_outproj, None))
        n_stages = len(stages)

        order = "new"
        for step in range(n_tiles + n_stages - 1):
            sis = range(n_stages) if order == "new" else range(n_stages - 1, -1, -1)
            for si in sis:
                ti = step - si
                if 0 <= ti < n_tiles:
                    if si == 0:
                        state[ti] = {}
                    fn, li = stages[si]
                    fn(ti, li)

        nc.sync.dma_start(out_d[:, :], osb[:, :])

    nc.compile()
    return nc


_CACHE = {}


def _get_nc(n_tiles, cfg=None):
    key = (n_tiles, tuple(sorted((cfg or CFG).items())))
    if key not in _CACHE:
        _CACHE[key] = build(n_tiles, cfg)
    return _CACHE[key]


def _prep_inputs(x, emb, ln1_w, ln1_b, ln2_w, ln2_b, ln3_w, ln3_b,
                 c1_w, c1_b, c2_w, c2_b, c3_w, c3_b, out_w, out_b):
    f32 = lambda a: np.ascontiguousarray(np.asarray(a), dtype=np.float32)
    rt = lambda a: np.ascontiguousarray(np.asarray(a, dtype=np.float32), dtype=NPRT)
    x = np.asarray(x)
    oneh = (x[:, None, :] == np.arange(VOCAB)[None, :, None]).astype(NPRT)

    c1_w, c2_w, c3_w = f32(c1_w), f32(c2_w), f32(c3_w)
    assert np.all(np.asarray(c1_b) == 0) and np.all(np.asarray(c2_b) == 0) \
        and np.all(np.asarray(c3_b) == 0), "conv biases assumed zero"
    for g in (ln1_w, ln2_w, ln3_w):
        assert np.all(np.asarray(g) == 1), "instance-norm gamma assumed one"
    for b in (ln1_b, ln2_b, ln3_b):
        assert np.all(np.asarray(b) == 0), "instance-norm beta assumed zero"
    assert np.all(np.asarray(out_b) == 0), "output bias assumed zero"

    w1h = rt(c1_w.transpose(0, 2, 1).reshape(NL, NCB_HI, 128, LOW))
    w2h = rt(c2_w.transpose(0, 3, 2, 1).reshape(NL, 5, NCB_LO, 128, LOW))
    w3h = rt(c3_w.transpose(0, 2, 1).reshape(NL, NCB_LO, 128, DIM))
    owh = rt(f32(out_w).reshape(VOCAB, NCB_HI, 128, K).transpose(1, 3, 2, 0))

    shared = {
        "embw": rt(emb), "w1": w1h, "w2": w2h, "w3": w3h, "ow": owh,
        "id1": np.eye(128, dtype=NPRT),
        "id9": (np.eye(128, dtype=np.float32) / 9.0).astype(NPRT),
    }
    in_maps = [{"oneh": np.ascontiguousarray(oneh[b]), **shared} for b in range(B)]
    return in_maps


def run(inputs, n_tiles=NT, n_cores=B, trace=False):
    nc = _get_nc(n_tiles)
    in_maps = _prep_inputs(**inputs)[:n_cores]
    res = run_bass_kernel_spmd(nc, in_maps, core_ids=list(range(n_cores)), trace=trace)
    out = np.stack([res.results[i]["out"].astype(np.float32).T
                    for i in range(n_cores)])
    return out, res


def kernel(**inputs):
    out, _ = run(inputs)
    return out.astype(np.float32)
